# revision 15
# baseline (speedup 1.0000x reference)
"""Trainium2 Bass kernel for nn_MoEGate_6150393168540 (moe_routing), v6.

Computes, for x [B=65536, D=1024], gate/expert weights [E=8, D] and biases [E]:
    gate = softmax(x @ gate_w.T + gate_b)            # [B, 8]
    keep top-k (k=2) gate values, zero the rest (no renormalization)
    expert = x @ expert_w.T + expert_b               # [B, 8]
    out = sum(gate_masked * expert, axis=1)          # [B, 1]

v6 strategy (8 NeuronCores, data-parallel over the batch):
  - Each core gets 8192 rows of x; weights replicated.
  - Scores need exact-fp32-grade logits (top-2 selection must order-match
    the reference), so the matmul uses the exact Dekker split
    hi = bf16-truncate(x), lo = fp16(x - hi); hi*(w_hi|w_lo) and lo*w_hi
    products are exact and accumulate in fp32 PSUM.
  - x is re-encoded on the host as the lossless (hi, lo) pair, already
    transposed into d-major tiles [DC, NG, 128, 2048]: same 4 B/elem of
    DMA, fully-contiguous 0.5 MB tiles, no device transposes.
  - Tiles round-robin between the sync and scalar HWDGE queues
    (parity of g*DC+c) so both queues carry 16.8 MB and finish together.
  - mm: lhsT = whl [128, 32] per chunk, rhs = tile slices [128, 512];
    four blocks share one PSUM tile [128, 512] via col-group
    tile_position.
  - Scores fold back to batch-major per GROUP via one fp32 PSUM->SBUF
    copy + fp32 masked-identity matmuls (exact: one nonzero * 1.0 per
    output), then a per-group postprocess split across DVE and GpSimd.
  - Postprocess output DMAs never share an engine with tile dma_starts
    (they'd head-of-line block the stream): mid-stream pairs go down
    gpsimd, the final pair down sync+scalar (idle by then).
  - Filler matmuls after the last real stage keep the PE active so the
    HAM clock gate stays at 8/8 through the tail fold+postprocess.
"""

import sys

sys.path.insert(0, "/opt/trn_rl_repo")

from contextlib import ExitStack

import numpy as np

import concourse.bass as bass
import concourse.mybir as mybir
import concourse.tile as tile

F32 = mybir.dt.float32
F16 = mybir.dt.float16
BF16 = mybir.dt.bfloat16
U16 = mybir.dt.uint16
ALU = mybir.AluOpType
AXX = mybir.AxisListType.X
EXP = mybir.ActivationFunctionType.Exp

B, D, E = 65536, 1024, 8
N_CORES = 8
B_LOC = B // N_CORES  # 8192
GB = 2048  # batch cols per group tile
NG = B_LOC // GB  # 4 groups
DC = D // 128  # 8 contraction chunks
N_WARM = 24  # dummy PE matmuls to release the HAM clock gate
PENDING = 5  # (c, b) stages of xh/xl lead the matmuls keep in flight
FOLD_DELAY = 6  # stages between a group's last matmul and its folds
PP_DELAY = 16  # stages between a group's last fold and its postprocess
FILL_COLS = 256  # filler matmul width (keeps the HAM clock gate at 8/8)
FILL_PER_STAGE = 2  # filler matmuls emitted after each real stage
N_TAIL_FILL = 28  # drain-phase fillers: hold the clock gate through the
#                   last fold+postprocess chain (it otherwise re-gates to
#                   half clock ~2us after the final real matmul)


def split_waits(nc, max_waits: int = 1) -> int:
    """walrus here allows only one semaphore wait per instruction; hoist the
    rest into preceding single-wait NOPs on the same engine (engine streams
    execute in order, so earlier waits on the same engine are equivalent)."""
    n_split = 0
    for f in nc.m.functions:
        for bb in f.blocks:
            new = []
            for inst in bb.instructions:
                si = inst.sync_info
                if si is not None and si.on_wait and len(si.on_wait) > max_waits:
                    waits = list(si.on_wait)
                    for w in waits[:-max_waits]:
                        n_split += 1
                        nop = mybir.InstNoOp(name=f"{inst.name}-ws{n_split}")
                        nop.engine = inst.engine
                        nop.sync_info = mybir.SyncInfo(on_wait=[w], on_update=[])
                        new.append(nop)
                    inst.sync_info = mybir.SyncInfo(
                        on_wait=waits[-max_waits:], on_update=list(si.on_update or [])
                    )
                new.append(inst)
            bb.instructions = new
    return n_split


def build_module(split: bool = True):
    nc = bass.Bass()
    xh = nc.dram_tensor("xh", [DC, NG, 128, GB], U16, kind="ExternalInput")
    xl = nc.dram_tensor("xl", [DC, NG, 128, GB], F16, kind="ExternalInput")
    whl = nc.dram_tensor("whl", [128, DC, 32], F16, kind="ExternalInput")
    brow = nc.dram_tensor("brow", [512], F32, kind="ExternalInput")
    foldg = nc.dram_tensor("foldg", [128, 4, 16], F32, kind="ExternalInput")
    y = nc.dram_tensor("y", [B_LOC], F32, kind="ExternalOutput")

    tt = nc.vector.tensor_tensor

    with tile.TileContext(nc) as tc, ExitStack() as ctx:
        consts = ctx.enter_context(tc.tile_pool(name="consts", bufs=1))
        xh_pool = ctx.enter_context(tc.tile_pool(name="xh", bufs=10))
        xl_pool = ctx.enter_context(tc.tile_pool(name="xl", bufs=10))
        zs_pool = ctx.enter_context(tc.tile_pool(name="zs", bufs=2))
        pp = ctx.enter_context(tc.tile_pool(name="pp", bufs=3))
        z_pool = ctx.enter_context(tc.tile_pool(name="zps", bufs=4, space="PSUM"))
        zt_pool = ctx.enter_context(tc.tile_pool(name="ztps", bufs=3, space="PSUM"))
        warm_pool = ctx.enter_context(tc.tile_pool(name="warm", bufs=1, space="PSUM"))

        whl_sb = consts.tile([128, DC, 32], F16)
        foldg_sb = consts.tile([128, 4, 16], F32)
        bias_sb = consts.tile([128, 512], F32)

        pair_state = {}

        def postprocess(g, zt_ps, out_engines):
            # zt_ps [128, 256] = [128 rows, 16 groups, 8 gate | 8 expert]
            # for batch rows 2048 g + 128 gg + p.
            zb = pp.tile([128, 16, 16], F32)
            nc.vector.tensor_add(
                zb, zt_ps.rearrange("p (g e) -> p g e", e=16),
                bias_sb[:, 0:256].rearrange("p (g e) -> p g e", e=16),
            )
            g8 = zb[:, :, 0:8]
            y8 = zb[:, :, 8:16]
            p8 = pp.tile([128, 16, 8], F32)
            nc.scalar.activation(p8, g8, EXP)
            # all elementwise work stays on DVE (the Pool engine's
            # TensorTensor fails the v3 ISA engine check at codegen)
            den = pp.tile([128, 16], F32)
            nc.vector.tensor_reduce(den, p8, axis=AXX, op=ALU.add)
            # reciprocal right after den so it hides under the tournament
            rden = pp.tile([128, 16], F32)
            nc.vector.reciprocal(rden, den)
            w8 = pp.tile([128, 16, 8], F32)
            tt(w8, p8, y8, op=ALU.mult)
            # top-2 threshold: tournament keeping (max, 2nd max) per segment
            h1 = pp.tile([128, 16, 4], F32)
            tt(h1, p8[:, :, 0:4], p8[:, :, 4:8], op=ALU.max)
            l1 = pp.tile([128, 16, 4], F32)
            tt(l1, p8[:, :, 0:4], p8[:, :, 4:8], op=ALU.min)
            h2 = pp.tile([128, 16, 2], F32)
            tt(h2, h1[:, :, 0:2], h1[:, :, 2:4], op=ALU.max)
            v2 = pp.tile([128, 16, 2], F32)
            tt(v2, h1[:, :, 0:2], h1[:, :, 2:4], op=ALU.min)
            u2 = pp.tile([128, 16, 2], F32)
            tt(u2, l1[:, :, 0:2], l1[:, :, 2:4], op=ALU.max)
            m2q = pp.tile([128, 16, 2], F32)
            tt(m2q, u2, v2, op=ALU.max)
            v3 = pp.tile([128, 16, 1], F32)
            tt(v3, h2[:, :, 0:1], h2[:, :, 1:2], op=ALU.min)
            u3 = pp.tile([128, 16, 1], F32)
            tt(u3, m2q[:, :, 0:1], m2q[:, :, 1:2], op=ALU.max)
            m2f = pp.tile([128, 16, 1], F32)
            tt(m2f, u3, v3, op=ALU.max)
            # mask & weighted sum
            msk = pp.tile([128, 16, 8], F32)
            tt(msk, p8, m2f.to_broadcast([128, 16, 8]), op=ALU.is_ge)
            prod = pp.tile([128, 16, 8], F32)
            tt(prod, msk, w8, op=ALU.mult)
            num = pp.tile([128, 16], F32)
            nc.vector.tensor_reduce(num, prod, axis=AXX, op=ALU.add)
            if g % 2 == 0:
                pair_state["outv"] = pp.tile(
                    [128, 32], F32, name="outv", tag="outv"
                )
            outv = pair_state["outv"]
            half = outv[:, 16 * (g % 2) : 16 * (g % 2) + 16]
            tt(half, num, rden, op=ALU.mult)
            if g % 2 == 1:
                # 32x32 block transpose: partition 32a+k of tv holds
                # y[b0 + 32a + 128 k + i] for i in 0..32
                tv = pp.tile([128, 32], F32)
                nc.vector.transpose(tv, outv)
                b0 = (g // 2) * 2 * GB
                yf = y.ap()
                for a in range(4):
                    # never on an engine that still has tile dma_starts
                    # queued: the wait on tv would head-of-line block them
                    dest = bass.AP(yf.tensor, b0 + 32 * a, [[128, 32], [1, 32]])
                    out_engines[a].dma_start(
                        out=dest, in_=tv[32 * a : 32 * a + 32, :]
                    )

        def finish_group(g, z_ps):
            # one exact fp32 copy of the group's scores out of PSUM, then
            # fp32 masked-identity folds to batch-major: output col
            # (4b+j)*16+e of zt gets score row (e | e+16) of block b,
            # batch col 128 j + o.  Each output has exactly one nonzero
            # product (* 1.0) per summed half, so fp32r rounding is moot.
            zs = zs_pool.tile([128, 512], F32, name="zs", tag="zs")
            nc.vector.tensor_copy(zs, z_ps)
            zt_ps = zt_pool.tile([128, 256], F32, name="zt_ps", tag="zt_ps")
            for j in range(4):
                for b in range(4):
                    col = (4 * b + j) * 16
                    nc.tensor.matmul(
                        zt_ps[:, col : col + 16],
                        zs[:, 128 * j : 128 * j + 128],
                        foldg_sb[:, b, :],
                        start=True, stop=True, skip_group_check=True,
                    )
            pp_pending.append((g, zt_ps, nstages[0]))

        def fill(n):
            # independent matmuls the PE can chew on while a real matmul
            # at the queue head waits on its tile DMA: without them a
            # >2us wait re-throttles the HAM clock gate to 1.2 GHz and
            # the PE can no longer keep up with the 420 GB/s stream
            for _ in range(n):
                nc.tensor.matmul(
                    wt_ps[:, 0:FILL_COLS], whl_sb[:, 0, :],
                    whl_flat[:, 0:FILL_COLS],
                    start=True, stop=True, skip_group_check=True,
                )

        def emit_stage(g, c, b, z_ps, xh_sb, xl_sb):
            tp = (0, 32 * b)
            out = z_ps[32 * b : 32 * b + 32, :]
            sl = slice(512 * b, 512 * b + 512)
            nc.tensor.matmul(
                out, whl_sb[:, c, :], xh_sb.bitcast(BF16)[:, sl],
                start=(c == 0), stop=False,
                tile_position=tp, skip_group_check=True,
            )
            nc.tensor.matmul(
                out, whl_sb[:, c, :], xl_sb[:, sl],
                start=False, stop=(c == DC - 1),
                tile_position=tp, skip_group_check=True,
            )
            fill(FILL_PER_STAGE)
            if c == DC - 1 and b == 3:
                fold_pending.append((g, z_ps, nstages[0]))

        # const DMAs first on the sync queue (so whl lands ~9us and the
        # warm matmuls release the clock gate before real work), bias
        # broadcast via gpsimd
        nc.sync.dma_start(out=whl_sb, in_=whl.ap())
        nc.sync.dma_start(out=foldg_sb, in_=foldg.ap())
        nc.gpsimd.dma_start(
            out=bias_sb, in_=brow.ap().unsqueeze(0).to_broadcast([128, 512])
        )

        # warm the PE (HAM clock gate releases after ~3.4us of activity):
        # dummy matmuls on the weights while the first x tiles are in flight
        wt_ps = warm_pool.tile([32, 256], F32, name="wt", tag="wt")
        whl_flat = whl_sb.rearrange("p c e -> p (c e)")
        for wi in range(N_WARM):
            nc.tensor.matmul(
                wt_ps, whl_sb[:, 0, :], whl_flat, start=True, stop=True,
                skip_group_check=True,
            )

        pending = []
        fold_pending = []
        pp_pending = []
        nstages = [0]

        def pop_stage():
            emit_stage(*pending.pop(0))
            nstages[0] += 1
            if fold_pending and nstages[0] - fold_pending[0][2] >= FOLD_DELAY:
                # the group's score copy has had FOLD_DELAY stages of lead
                # time; its folds won't make the PE wait
                g, zp, _ = fold_pending.pop(0)
                finish_group(g, zp)
            if pp_pending and nstages[0] - pp_pending[0][2] >= PP_DELAY:
                # by now the zt folds are done; the postprocess ops (incl.
                # the exp on the ACT queue) won't stall the xl tile DMAs.
                # mid-stream output DMAs go to the otherwise-idle gpsimd
                # queue so nothing real queues behind their long waits
                g, zt, _ = pp_pending.pop(0)
                postprocess(g, zt, [nc.gpsimd] * 4)

        for g in range(NG):
            z_ps = z_pool.tile([128, 512], F32)
            for c in range(DC):
                xh_sb = xh_pool.tile([128, GB], U16, name="xh_sb", tag="xh_sb")
                xl_sb = xl_pool.tile([128, GB], F16, name="xl_sb", tag="xl_sb")
                # round-robin the two tile DMAs over the two hardware
                # queues so both carry the same byte total
                if (g * DC + c) % 2 == 0:
                    nc.sync.dma_start(out=xh_sb, in_=xh.ap()[c, g])
                    nc.scalar.dma_start(out=xl_sb, in_=xl.ap()[c, g])
                else:
                    nc.scalar.dma_start(out=xh_sb, in_=xh.ap()[c, g])
                    nc.sync.dma_start(out=xl_sb, in_=xl.ap()[c, g])
                for b in range(4):
                    pending.append((g, c, b, z_ps, xh_sb, xl_sb))
                    if len(pending) > PENDING:
                        pop_stage()
        while pending:
            pop_stage()
        # drain in dependency-friendly order: non-final folds and pps
        # first (their inputs are long since ready), so the only work
        # after the last tile is the final group's fold+postprocess
        while fold_pending and fold_pending[0][0] < NG - 1:
            g, zp, _ = fold_pending.pop(0)
            finish_group(g, zp)
        while pp_pending and pp_pending[0][0] < NG - 1:
            g, zt, _ = pp_pending.pop(0)
            postprocess(g, zt, [nc.gpsimd] * 4)
        while fold_pending:
            g, zp, _ = fold_pending.pop(0)
            finish_group(g, zp)
        while pp_pending:
            # tail postprocess: sync + scalar hardware queues are idle by
            # now, so the four output DMAs issue two-by-two in parallel
            g, zt, _ = pp_pending.pop(0)
            postprocess(g, zt, [nc.sync, nc.scalar, nc.sync, nc.scalar])
        # independent fillers so the PE stays active (and the HAM clock
        # gate stays at 8/8) while the tail fold+postprocess chain runs on
        # the vector/act/gpsimd engines; sized to end with the postprocess
        fill(N_TAIL_FILL)

    if split:
        split_waits(nc)
    return nc


def host_inputs(gate_w, gate_b, expert_w, expert_b):
    """Host-side prep of the small replicated tensors."""
    W = np.concatenate([gate_w, expert_w], axis=0).astype(np.float32)  # [16, D]
    WT = W.T  # [D, 16]
    w_hi = WT.astype(np.float16)
    w_lo = (WT - w_hi.astype(np.float32)).astype(np.float16)
    whl = np.empty((128, DC, 32), dtype=np.float16)
    for c in range(DC):
        whl[:, c, 0:16] = w_hi[128 * c : 128 * (c + 1), :]
        whl[:, c, 16:32] = w_lo[128 * c : 128 * (c + 1), :]
    bcat = np.concatenate([gate_b, expert_b]).astype(np.float32)  # [16]
    brow = np.tile(bcat, 32)  # [512]
    foldg = np.zeros((128, 4, 16), dtype=np.float32)
    eye = np.eye(16, dtype=np.float32)
    for b in range(4):
        foldg[32 * b : 32 * b + 16, b] = eye
        foldg[32 * b + 16 : 32 * b + 32, b] = eye
    return {"whl": whl, "brow": brow, "foldg": foldg}


_NC_CACHE = {}


def kernel(x, gate_w, gate_b, expert_w, expert_b, k):
    assert int(k) == 2
    x = np.ascontiguousarray(np.asarray(x, dtype=np.float32))
    assert x.shape == (B, D)

    from concourse.bass_utils import run_bass_kernel_spmd

    if "v6" not in _NC_CACHE:
        _NC_CACHE["v6"] = build_module()
    nc = _NC_CACHE["v6"]

    common = host_inputs(
        np.asarray(gate_w, np.float32),
        np.asarray(gate_b, np.float32),
        np.asarray(expert_w, np.float32),
        np.asarray(expert_b, np.float32),
    )
    in_maps = []
    for i in range(N_CORES):
        xc = x[i * B_LOC : (i + 1) * B_LOC]
        # [c, g, p, j] <- x[g*2048 + j, c*128 + p]
        xt = np.ascontiguousarray(
            xc.reshape(NG, GB, DC, 128).transpose(2, 0, 3, 1)
        )
        # lossless Dekker re-encoding (same prep as the weights get):
        # hi = bf16-truncate (high u16 of each fp32), lo = fp16(x - hi)
        xth = np.ascontiguousarray(xt.view(np.uint16)[..., 1::2])
        hi_f32 = (xth.astype(np.uint32) << 16).view(np.float32)
        xtl = (xt - hi_f32).astype(np.float16)
        in_maps.append({**common, "xh": xth, "xl": xtl})
    import os

    trace = bool(os.environ.get("MOE_TRACE"))
    if trace:
        _ensure_ntff_hook()
    res = run_bass_kernel_spmd(
        nc, in_maps, core_ids=list(range(N_CORES)), trace=trace
    )
    global LAST_RESULT
    LAST_RESULT = res
    out = np.concatenate([r["y"] for r in res.results])
    return out.reshape(B, 1).astype(np.float32)


LAST_RESULT = None


def _ensure_ntff_hook():
    """Register the axon NTFF profile hook if the antenv shim is missing
    (lets run_bass_kernel_spmd(trace=True) capture HW timing)."""
    try:
        import antenv.axon_hooks  # noqa: F401

        return
    except ImportError:
        pass
    try:
        import types

        import antenv
        from trn_agent_boot.trn_boot import _ntff_profile_via_ctypes

        mod = types.ModuleType("antenv.axon_hooks")
        _h = [None]
        mod.set_axon_ntff_profile_hook = lambda h: _h.__setitem__(0, h)
        mod.get_axon_ntff_profile_hook = lambda: _h[0]
        sys.modules["antenv.axon_hooks"] = mod
        antenv.axon_hooks = mod
        mod.set_axon_ntff_profile_hook(
            _ntff_profile_via_ctypes("/opt/axon/libaxon_pjrt.so")
        )
    except Exception as e:  # profiling is best-effort
        print(f"ntff hook setup failed: {e}")


if __name__ == "__main__":
    rng = np.random.default_rng(0)
    s = 1.0 / np.sqrt(D)
    inputs = {
        "x": rng.standard_normal((B, D), dtype=np.float32),
        "gate_w": rng.uniform(-s, s, (E, D)).astype(np.float32),
        "gate_b": rng.uniform(-s, s, E).astype(np.float32),
        "expert_w": rng.uniform(-s, s, E).astype(np.float32),
        "expert_b": rng.uniform(-s, s, E).astype(np.float32),
        "k": 2,
    }
    got = kernel(**inputs)
    print("kernel output:", got.shape, got.dtype, got[:4, 0])
